# revision 3
# baseline (speedup 1.0000x reference)
"""Trainium2 Bass kernel for nn_CrossAttentionModule (head-collapsed cross attention).

Math (reference):
    Q = x @ Wq.T ; K = y @ Wk.T ; V = y @ Wv.T          (torch Linear convention)
    energy[n,q,k] = sum_{h,d} Q[n,q,h,d] K[n,k,h,d]     (heads summed!)
    att = softmax(energy / sqrt(512), axis=k)
    out = x + (att @ V) @ Wo.T + bo

Because heads are summed, energy = x @ (Wq.T @ Wk) @ y.T and the output
projection folds into V:  (att @ V) @ Wo.T = att @ (y @ (Wo @ Wv).T).
So we precompute on host (512x512, trivial):
    A    = Wq.T @ Wk        -> energy = (x @ A) @ y.T
    WvoT = Wv.T @ Wo.T      -> Vp = y @ WvoT ; att_out = att @ Vp
Device (per core, data-parallel over the N=8 batch), all fp8 DoubleRow:
    per q-block qb (512 wide):
      tT = A.T @ xT          [e2, qb]  (prologue, 8 MMs)
      kp loop (depth-2 software pipeline; on qb==0 also Vp[kt] = y @ WvoT):
        S^T tile = yT.T @ tT [k, q]   fp32 psum  (k on partitions)
        P = exp(S^T/sqrt(512) - C)    fp8
        att_psum += P.T @ Vp [q, f]   accumulated over kp
        den_tmp   = P.T @ ones        (transient psum; DVE-accumulated in SBUF)
      out_bf16 = att_psum * (1/den)  -> DRAM (per 128-row slice)
Host adds the residual x + out + bo in fp32.
"""

import sys

sys.path.insert(0, "/opt/trn_rl_repo")

import ml_dtypes
import numpy as np

import bass_rust
import concourse.bass as bass
import concourse.mybir as mybir
import concourse.tile as tile
from concourse.bass_utils import run_bass_kernel_spmd
from concourse.vector_clock import ScopedClock

N_CORES = 8
E = 512  # embed dim
Q = 2048  # query length (per batch element)
K = 4096  # key/value length
P = 128  # partitions
ET = E // P  # 4 embed tiles
QB = 512  # q block width for S^T matmuls
NQB = Q // QB  # 4
QS = P  # q sub-block (att psum partition dim)
NQS = QB // QS  # 4
KT = K // P  # 32 k tiles
KP = KT // 2  # 16 k-pair tiles (fp8 DoubleRow)
PIPE = 2  # kp-loop software pipeline depth
SCALE = float(1.0 / np.sqrt(np.float32(512.0)))
# exp shift: P' = exp(s/sqrt(512) - C) fits e4m3 (max logit ~8.1 -> P' <= 62);
# the flushed tail (weights < 2^-9 of e^C) carries ~1e-3 of the softmax mass.
C_SHIFT = 4.0

BF16 = mybir.dt.bfloat16
F32 = mybir.dt.float32
FP8E4 = mybir.dt.float8e4
BF16_NP = ml_dtypes.bfloat16
E4_NP = ml_dtypes.float8_e4m3


def _patched_drain_and_barrier(self, tick_clock, wait_clock):
    # The walrus build in this container caps sync-wait commands per CTRL
    # instruction below what Tile's tail drain emits; split the waits across
    # separate SP nops (same engine => same ordering semantics).
    nc = self.nc
    probe = nc.sync.nop(nofuse=True)
    wait_clock.add_sem_waits(probe.ins, ScopedClock({None: tick_clock.global_clock}))
    waits = list(probe.ins.sync_info.on_wait)
    probe.ins.sync_info = bass_rust.SyncInfo(on_wait=waits[:1], on_update=[])
    for wval in waits[1:]:
        n2 = nc.sync.nop(nofuse=True)
        n2.ins.sync_info = bass_rust.SyncInfo(on_wait=[wval], on_update=[])
    nc.sync.drain()
    nc.all_engine_barrier()
    popped = nc._tile_sem_poison_stack.pop()
    assert popped is self._sem_poison
    # Inline clear_and_free_semaphores, but spread the sem clears over all
    # engines (they serialize ~30ns each; ~250 sems on one engine is ~7us of
    # tail). dma_reset must stay on gpsimd. No trailing all_engine_barrier:
    # NEFF completion waits for every engine to halt anyway, so the next
    # execution still sees cleared semaphores.
    from concourse.bass import compact_to_ranges

    sems = list(self.sems.allocated().values())
    if sems:
        sem_nums = [s.num if hasattr(s, "num") else s for s in sems]
        engines = [nc.gpsimd, nc.vector, nc.scalar, nc.tensor, nc.sync]
        for sem_range in compact_to_ranges(sem_nums):
            assert nc._state.free_isdisjoint(sem_range)
            nc.gpsimd.dma_reset(sem_range)
            n = len(sem_range)
            n_eng = len(engines)
            step = (n + n_eng - 1) // n_eng
            for ei, lo in enumerate(range(0, n, step)):
                sub = range(sem_range.start + lo, sem_range.start + min(lo + step, n))
                engines[ei % n_eng].sem_clear(sub)
        nc._state.prepend_free_semaphores(sem_nums)
        for poison_set in nc._tile_sem_poison_stack:
            poison_set.update(sem_nums)


tile.TileContext._drain_and_barrier = _patched_drain_and_barrier

_MAX_WAITS = 1  # walrus merges Ldweights+Matmult waits into one struct capped at 2


def _split_sync_waits(nc, max_waits=_MAX_WAITS):
    # Hoist sem waits beyond the per-instruction cap onto same-engine NoOps
    # inserted right before the offender (same engine => same order semantics).
    # For Matmult preceded by its Ldweights, nops go before the Ldweights so
    # walrus can still fuse the pair (their waits are summed in the MM struct).
    n_nops = 0
    for f in nc.m.functions:
        for bb in f.blocks:
            new_insts = []
            changed = False
            for inst in bb.instructions:
                si = getattr(inst, "sync_info", None)
                waits = list(si.on_wait) if si is not None else []
                if len(waits) > max_waits:
                    head, rest = waits[:-max_waits], waits[-max_waits:]
                    pos = len(new_insts)
                    if (
                        isinstance(inst, mybir.InstMatmult)
                        and new_insts
                        and isinstance(new_insts[-1], mybir.InstLdweights)
                    ):
                        pos -= 1
                    nops = []
                    for i0 in range(0, len(head), max_waits):
                        nops.append(
                            mybir.InstNoOp(
                                name=f"{inst.name}-wsplit{i0}",
                                sync_info=mybir.SyncInfo(
                                    on_wait=head[i0 : i0 + max_waits], on_update=[]
                                ),
                                bass_nofuse=True,
                                engine=inst.engine,
                            )
                        )
                        n_nops += 1
                    new_insts[pos:pos] = nops
                    inst.sync_info = mybir.SyncInfo(
                        on_wait=rest, on_update=list(si.on_update)
                    )
                    changed = True
                new_insts.append(inst)
            if changed:
                bb.instructions = new_insts
    return n_nops


def _build_fp8():
    """fp8 DoubleRow variant: contraction dims pair-packed as [128, 2, n].

    Pair layout: virtual contraction row (pair, p, i) = index pair*256 + i*128 + p.
    lhsT and rhs use the same (p, i) mapping, so the DoubleRow pairing is
    consistent regardless of the hardware's internal interleave order.
    """
    nc = bass.Bass()
    x8 = nc.dram_tensor("x8", [2, P, 2, Q], FP8E4, kind="ExternalInput")
    y8 = nc.dram_tensor("y8", [2, P, 2, K], FP8E4, kind="ExternalInput")
    A8 = nc.dram_tensor("A8", [2, P, 2, E], FP8E4, kind="ExternalInput")
    Wvo8 = nc.dram_tensor("Wvo8", [2, P, 2, E], FP8E4, kind="ExternalInput")
    out = nc.dram_tensor("out", [Q, E], BF16, kind="ExternalOutput")

    exp = mybir.ActivationFunctionType.Exp
    DR = mybir.MatmulPerfMode.DoubleRow

    with tile.TileContext(nc) as tc:
        with (
            tc.tile_pool(name="const", bufs=1) as cpool,
            tc.tile_pool(name="pwork", bufs=4) as wpool,
            tc.tile_pool(name="outp", bufs=4) as opool,
            tc.tile_pool(name="ps_mm", bufs=4, space="PSUM") as ps_mm,
            tc.tile_pool(name="ps_att", bufs=1, space="PSUM") as ps_att,
        ):
            x8_sb = [cpool.tile([P, 2, Q], FP8E4, name=f"x8{i}") for i in range(2)]
            A8_sb = [cpool.tile([P, 2, E], FP8E4, name=f"A8{i}") for i in range(2)]
            y8_sb = [cpool.tile([P, 2, K], FP8E4, name=f"y8{i}") for i in range(2)]
            Wv8_sb = [cpool.tile([P, 2, E], FP8E4, name=f"Wv8{i}") for i in range(2)]
            t8_sb = [cpool.tile([P, 2, Q], FP8E4, name=f"t8{i}") for i in range(2)]
            Vp8_sb = [cpool.tile([P, 2, E], FP8E4, name=f"Vp8{i}") for i in range(KP)]
            ones_sb = cpool.tile([P, 32], FP8E4, name="ones")
            nc.vector.memset(ones_sb[:], 1.0)
            bias_sb = cpool.tile([P, 1], F32, name="biasC")
            nc.vector.memset(bias_sb[:], -C_SHIFT)
            # rhs AP [128, 2, 1] with middle step 16 (DoubleRow needs step%16==0)
            ones_ap = ones_sb.rearrange("p (i c) -> p i c", c=16)[:, :, 0:1]

            # Input DMAs spread across engine queues, issued in consumption
            # order: A8 + x8-h0 gate the first prologue, y8 quarter 0 + Wv8
            # gate the first kp iteration; the rest stream in behind.
            HQ = Q // 2  # 1024
            for i in range(2):
                nc.scalar.dma_start(A8_sb[i][:], A8[i])
            for i in range(2):
                nc.sync.dma_start(x8_sb[i][:, :, 0:HQ], x8[i][:, :, 0:HQ])
            for i in range(2):
                nc.scalar.dma_start(x8_sb[i][:, :, HQ:Q], x8[i][:, :, HQ:Q])
            QK = K // 4  # 1024
            for i in range(2):
                nc.gpsimd.dma_start(y8_sb[i][:, :, 0:QK], y8[i][:, :, 0:QK])
            for i in range(2):
                nc.gpsimd.dma_start(Wv8_sb[i][:], Wvo8[i])
            for qtr in range(1, 4):
                for i in range(2):
                    nc.gpsimd.dma_start(
                        y8_sb[i][:, :, qtr * QK : (qtr + 1) * QK],
                        y8[i][:, :, qtr * QK : (qtr + 1) * QK],
                    )

            for qb in range(NQB):
                # Phase-1 prologue for this q block: tT[e2, qb] = A.T @ x
                for e2 in range(ET):
                    pt = ps_mm.tile([P, QB], F32, name="ps_s")
                    for pr in range(2):
                        nc.tensor.matmul(
                            pt[:],
                            A8_sb[pr][:, :, e2 * P : (e2 + 1) * P],
                            x8_sb[pr][:, :, qb * QB : (qb + 1) * QB],
                            start=(pr == 0),
                            stop=(pr == 1),
                            perf_mode=DR,
                        )
                    nc.vector.tensor_copy(
                        t8_sb[e2 // 2][:, e2 % 2, qb * QB : (qb + 1) * QB], pt[:]
                    )

                att_ps = [ps_att.tile([P, E], F32, name=f"att{j}") for j in range(NQS)]
                den_sb = opool.tile([P, NQS], F32, name="den_sb")
                p8_tiles = [None] * KP
                for i in range(KP + PIPE):
                    if i < KP:
                        p8 = wpool.tile([P, 2, QB], FP8E4, name="p8")
                        p8_tiles[i] = p8
                        for half in range(2):
                            kt = 2 * i + half
                            st = ps_mm.tile([P, QB], F32, name="ps_s")
                            for pr in range(2):
                                nc.tensor.matmul(
                                    st[:],
                                    y8_sb[pr][:, :, kt * P : (kt + 1) * P],
                                    t8_sb[pr][:, :, qb * QB : (qb + 1) * QB],
                                    start=(pr == 0),
                                    stop=(pr == 1),
                                    perf_mode=DR,
                                )
                            nc.scalar.activation(
                                p8[:, half, :], st[:], exp, bias=bias_sb[:], scale=SCALE
                            )
                        if qb == 0:
                            # Vp[kt] = y @ WvoT for this kp, interleaved so the
                            # PE never sits idle in a separate phase
                            for half in range(2):
                                kt = 2 * i + half
                                pv = ps_mm.tile([P, E], F32, name="ps_s")
                                for pr in range(2):
                                    nc.tensor.matmul(
                                        pv[:],
                                        y8_sb[pr][:, :, kt * P : (kt + 1) * P],
                                        Wv8_sb[pr][:],
                                        start=(pr == 0),
                                        stop=(pr == 1),
                                        perf_mode=DR,
                                    )
                                nc.vector.tensor_copy(Vp8_sb[i][:, half, :], pv[:])
                    if i >= PIPE:
                        kp = i - PIPE
                        p8p = p8_tiles[kp]
                        p8_tiles[kp] = None
                        den_ps = ps_mm.tile([P, NQS], F32, name="ps_s")
                        for j in range(NQS):
                            nc.tensor.matmul(
                                att_ps[j][:],
                                p8p[:, :, j * QS : (j + 1) * QS],
                                Vp8_sb[kp][:],
                                start=(kp == 0),
                                stop=(kp == KP - 1),
                                perf_mode=DR,
                            )
                            nc.tensor.matmul(
                                den_ps[:, j : j + 1],
                                p8p[:, :, j * QS : (j + 1) * QS],
                                ones_ap,
                                start=True,
                                stop=True,
                                perf_mode=DR,
                            )
                        if kp == 0:
                            nc.vector.tensor_copy(den_sb[:], den_ps[:])
                        else:
                            nc.vector.tensor_add(den_sb[:], den_sb[:], den_ps[:])

                rec_sb = opool.tile([P, NQS], F32, name="rec")
                nc.vector.reciprocal(rec_sb[:], den_sb[:])
                for j in range(NQS):
                    o_sb = opool.tile([P, E], BF16, name="osb")
                    # alternate DVE/ACT so the last q-block's epilogue halves
                    if j % 2 == 0:
                        nc.vector.tensor_scalar_mul(
                            o_sb[:], att_ps[j][:], rec_sb[:, j : j + 1]
                        )
                        eng = nc.sync
                    else:
                        nc.scalar.mul(o_sb[:], att_ps[j][:], rec_sb[:, j : j + 1])
                        eng = nc.gpsimd
                    eng.dma_start(
                        out[qb * QB + j * QS : qb * QB + (j + 1) * QS, :], o_sb[:]
                    )

    _split_sync_waits(nc)
    return nc


_CACHED_NC = None


def _get_nc():
    global _CACHED_NC
    if _CACHED_NC is None:
        _CACHED_NC = _build_fp8()
    return _CACHED_NC


def _pair_pack(m):
    # [512, n] -> [2, 128, 2, n] with (pair, p, i) -> row pair*256 + i*128 + p
    n = m.shape[1]
    return np.ascontiguousarray(m.reshape(2, 2, P, n).transpose(0, 2, 1, 3))


def _prep_inputs(x, y, Wq, Wk, Wv, Wo):
    A8 = _pair_pack((Wq.T @ Wk).astype(E4_NP))
    WvoT8 = _pair_pack((Wv.T @ Wo.T).astype(E4_NP))
    x8 = np.stack([_pair_pack(x[n].T.astype(E4_NP)) for n in range(N_CORES)])
    y8 = np.stack([_pair_pack(y[n].T.astype(E4_NP)) for n in range(N_CORES)])
    return [
        {"x8": x8[n], "y8": y8[n], "A8": A8, "Wvo8": WvoT8} for n in range(N_CORES)
    ]


def run_device(x, y, Wq, Wk, Wv, Wo, **spmd_kwargs):
    nc = _get_nc()
    in_maps = _prep_inputs(x, y, Wq, Wk, Wv, Wo)
    res = run_bass_kernel_spmd(nc, in_maps, core_ids=list(range(N_CORES)), **spmd_kwargs)
    att = np.stack(
        [np.asarray(res.results[n]["out"]).astype(np.float32) for n in range(N_CORES)]
    )
    return att, res


def kernel(x, y, Wq, Wk, Wv, Wo, bo):
    x = np.asarray(x, dtype=np.float32)
    y = np.asarray(y, dtype=np.float32)
    Wq = np.asarray(Wq, dtype=np.float32)
    Wk = np.asarray(Wk, dtype=np.float32)
    Wv = np.asarray(Wv, dtype=np.float32)
    Wo = np.asarray(Wo, dtype=np.float32)
    bo = np.asarray(bo, dtype=np.float32)
    att, _ = run_device(x, y, Wq, Wk, Wv, Wo)
    return x + att + bo[None, None, :]


# revision 8
# speedup vs baseline: 1.1087x; 1.1087x over previous
"""Trainium2 Bass kernel for nn_CrossAttentionModule (head-collapsed cross attention).

Math (reference):
    Q = x @ Wq.T ; K = y @ Wk.T ; V = y @ Wv.T          (torch Linear convention)
    energy[n,q,k] = sum_{h,d} Q[n,q,h,d] K[n,k,h,d]     (heads summed!)
    att = softmax(energy / sqrt(512), axis=k)
    out = x + (att @ V) @ Wo.T + bo

Because heads are summed, energy = x @ (Wq.T @ Wk) @ y.T and the output
projection folds into V:  (att @ V) @ Wo.T = att @ (y @ (Wo @ Wv).T).
So we precompute on host (512x512, trivial):
    A    = Wq.T @ Wk        -> energy = (x @ A) @ y.T
    WvoT = Wv.T @ Wo.T      -> Vp = y @ WvoT ; att_out = att @ Vp
Device (per core, data-parallel over the N=8 batch), all fp8 DoubleRow:
    per q-block qb (512 wide):
      tT = A.T @ xT          [e2, qb]  (prologue, 8 MMs)
      kp loop (depth-2 software pipeline; on qb==0 also Vp[kt] = y @ WvoT):
        S^T tile = yT.T @ tT [k, q]   fp32 psum  (k on partitions)
        P = exp(S^T/sqrt(512) - C)    fp8
        att_psum += P.T @ Vp [q, f]   accumulated over kp
        den_tmp   = P.T @ ones        (transient psum; DVE-accumulated in SBUF)
      out_bf16 = att_psum * (1/den)  -> DRAM (per 128-row slice)
Host adds the residual x + out + bo in fp32.
"""

import sys

sys.path.insert(0, "/opt/trn_rl_repo")

import ml_dtypes
import numpy as np

import bass_rust
import concourse.bass as bass
import concourse.mybir as mybir
import concourse.tile as tile
from concourse.bass_utils import run_bass_kernel_spmd
from concourse.vector_clock import ScopedClock

N_CORES = 8
E = 512  # embed dim
Q = 2048  # query length (per batch element)
K = 4096  # key/value length
P = 128  # partitions
ET = E // P  # 4 embed tiles
QB = 512  # q block width for S^T matmuls
NQB = Q // QB  # 4
QS = P  # q sub-block (att psum partition dim)
NQS = QB // QS  # 4
KT = K // P  # 32 k tiles
KP = KT // 2  # 16 k-pair tiles (fp8 DoubleRow)
PIPE = 2  # kp-loop software pipeline depth
SCALE = float(1.0 / np.sqrt(np.float32(512.0)))
# exp shift: P' = exp(s/sqrt(512) - C) fits e4m3 (max logit ~8.1 -> P' <= 62);
# the flushed tail (weights < 2^-9 of e^C) carries ~1e-3 of the softmax mass.
C_SHIFT = 4.0

BF16 = mybir.dt.bfloat16
F32 = mybir.dt.float32
FP8E4 = mybir.dt.float8e4
BF16_NP = ml_dtypes.bfloat16
E4_NP = ml_dtypes.float8_e4m3


def _patched_drain_and_barrier(self, tick_clock, wait_clock):
    # The walrus build in this container caps sync-wait commands per CTRL
    # instruction below what Tile's tail drain emits; split the waits across
    # separate SP nops (same engine => same ordering semantics).
    nc = self.nc
    probe = nc.sync.nop(nofuse=True)
    wait_clock.add_sem_waits(probe.ins, ScopedClock({None: tick_clock.global_clock}))
    waits = list(probe.ins.sync_info.on_wait)
    probe.ins.sync_info = bass_rust.SyncInfo(on_wait=waits[:1], on_update=[])
    for wval in waits[1:]:
        n2 = nc.sync.nop(nofuse=True)
        n2.ins.sync_info = bass_rust.SyncInfo(on_wait=[wval], on_update=[])
    nc.sync.drain()
    nc.all_engine_barrier()
    popped = nc._tile_sem_poison_stack.pop()
    assert popped is self._sem_poison
    # Inline clear_and_free_semaphores, but spread the sem clears over all
    # engines (they serialize ~30ns each; ~250 sems on one engine is ~7us of
    # tail). dma_reset must stay on gpsimd. No trailing all_engine_barrier:
    # NEFF completion waits for every engine to halt anyway, so the next
    # execution still sees cleared semaphores.
    from concourse.bass import compact_to_ranges

    sems = list(self.sems.allocated().values())
    if sems:
        sem_nums = [s.num if hasattr(s, "num") else s for s in sems]
        engines = [nc.gpsimd, nc.vector, nc.scalar, nc.tensor, nc.sync]
        for sem_range in compact_to_ranges(sem_nums):
            assert nc._state.free_isdisjoint(sem_range)
            nc.gpsimd.dma_reset(sem_range)
            n = len(sem_range)
            n_eng = len(engines)
            step = (n + n_eng - 1) // n_eng
            for ei, lo in enumerate(range(0, n, step)):
                sub = range(sem_range.start + lo, sem_range.start + min(lo + step, n))
                engines[ei % n_eng].sem_clear(sub)
        nc._state.prepend_free_semaphores(sem_nums)
        for poison_set in nc._tile_sem_poison_stack:
            poison_set.update(sem_nums)


tile.TileContext._drain_and_barrier = _patched_drain_and_barrier

_MAX_WAITS = 1  # walrus merges Ldweights+Matmult waits into one struct capped at 2


def _split_sync_waits(nc, max_waits=_MAX_WAITS):
    # Hoist sem waits beyond the per-instruction cap onto same-engine NoOps
    # inserted right before the offender (same engine => same order semantics).
    # For Matmult preceded by its Ldweights, nops go before the Ldweights so
    # walrus can still fuse the pair (their waits are summed in the MM struct).
    n_nops = 0
    for f in nc.m.functions:
        for bb in f.blocks:
            new_insts = []
            changed = False
            for inst in bb.instructions:
                si = getattr(inst, "sync_info", None)
                waits = list(si.on_wait) if si is not None else []
                if len(waits) > max_waits:
                    head, rest = waits[:-max_waits], waits[-max_waits:]
                    pos = len(new_insts)
                    if (
                        isinstance(inst, mybir.InstMatmult)
                        and new_insts
                        and isinstance(new_insts[-1], mybir.InstLdweights)
                    ):
                        pos -= 1
                    nops = []
                    for i0 in range(0, len(head), max_waits):
                        nops.append(
                            mybir.InstNoOp(
                                name=f"{inst.name}-wsplit{i0}",
                                sync_info=mybir.SyncInfo(
                                    on_wait=head[i0 : i0 + max_waits], on_update=[]
                                ),
                                bass_nofuse=True,
                                engine=inst.engine,
                            )
                        )
                        n_nops += 1
                    new_insts[pos:pos] = nops
                    inst.sync_info = mybir.SyncInfo(
                        on_wait=rest, on_update=list(si.on_update)
                    )
                    changed = True
                new_insts.append(inst)
            if changed:
                bb.instructions = new_insts
    return n_nops


def _build_fp8():
    """fp8 DoubleRow variant: contraction dims pair-packed as [128, 2, n].

    Pair layout: virtual contraction row (pair, p, i) = index pair*256 + i*128 + p.
    lhsT and rhs use the same (p, i) mapping, so the DoubleRow pairing is
    consistent regardless of the hardware's internal interleave order.
    """
    nc = bass.Bass()
    x8 = nc.dram_tensor("x8", [2, P, 2, Q], FP8E4, kind="ExternalInput")
    y8 = nc.dram_tensor("y8", [2, P, 2, K], FP8E4, kind="ExternalInput")
    A8 = nc.dram_tensor("A8", [2, P, 2, E], FP8E4, kind="ExternalInput")
    Wvo8 = nc.dram_tensor("Wvo8", [2, P, 2, E], FP8E4, kind="ExternalInput")
    out = nc.dram_tensor("out", [Q, E], BF16, kind="ExternalOutput")

    exp = mybir.ActivationFunctionType.Exp
    DR = mybir.MatmulPerfMode.DoubleRow

    with tile.TileContext(nc) as tc:
        with (
            tc.tile_pool(name="const", bufs=1) as cpool,
            tc.tile_pool(name="pwork", bufs=4) as wpool,
            tc.tile_pool(name="outp", bufs=4) as opool,
            tc.tile_pool(name="ps_mm", bufs=3, space="PSUM") as ps_mm,
            tc.tile_pool(name="ps_att", bufs=1, space="PSUM") as ps_att,
            tc.tile_pool(name="ps_den", bufs=1, space="PSUM") as ps_den,
        ):
            x8_sb = [cpool.tile([P, 2, Q], FP8E4, name=f"x8{i}") for i in range(2)]
            A8_sb = [cpool.tile([P, 2, E], FP8E4, name=f"A8{i}") for i in range(2)]
            y8_sb = [cpool.tile([P, 2, K], FP8E4, name=f"y8{i}") for i in range(2)]
            Wv8_sb = [cpool.tile([P, 2, E], FP8E4, name=f"Wv8{i}") for i in range(2)]
            t8_sb = [cpool.tile([P, 2, Q], FP8E4, name=f"t8{i}") for i in range(2)]
            Vp8_sb = [cpool.tile([P, 2, E], FP8E4, name=f"Vp8{i}") for i in range(KP)]
            ones_sb = cpool.tile([P, 32], FP8E4, name="ones")
            nc.vector.memset(ones_sb[:], 1.0)
            bias_sb = cpool.tile([P, 1], F32, name="biasC")
            nc.vector.memset(bias_sb[:], -C_SHIFT)
            # rhs AP [128, 2, 1] with middle step 16 (DoubleRow needs step%16==0)
            ones_ap = ones_sb.rearrange("p (i c) -> p i c", c=16)[:, :, 0:1]

            # Input DMAs spread across engine queues, issued in consumption
            # order. First wave is only what gates the first prologue matmul
            # (A8 + the qb0 slice of x8, 512KB total, one DMA per queue so
            # descriptor generation is parallel); y8 q0 + Wv8 follow for the
            # first kp iteration; the rest stream in behind.
            QK = K // 4  # 1024
            nc.scalar.dma_start(A8_sb[0][:], A8[0])
            nc.sync.dma_start(A8_sb[1][:], A8[1])
            nc.gpsimd.dma_start(x8_sb[0][:, :, 0:QB], x8[0][:, :, 0:QB])
            nc.gpsimd.dma_start(x8_sb[1][:, :, 0:QB], x8[1][:, :, 0:QB])
            nc.scalar.dma_start(Wv8_sb[0][:], Wvo8[0])
            nc.sync.dma_start(Wv8_sb[1][:], Wvo8[1])
            for i in range(2):
                nc.gpsimd.dma_start(y8_sb[i][:, :, 0:QK], y8[i][:, :, 0:QK])
            for qtr in range(1, 4):
                nc.scalar.dma_start(
                    y8_sb[0][:, :, qtr * QK : (qtr + 1) * QK],
                    y8[0][:, :, qtr * QK : (qtr + 1) * QK],
                )
                nc.sync.dma_start(
                    y8_sb[1][:, :, qtr * QK : (qtr + 1) * QK],
                    y8[1][:, :, qtr * QK : (qtr + 1) * QK],
                )
            for i in range(2):
                nc.gpsimd.dma_start(x8_sb[i][:, :, QB : 2 * QB], x8[i][:, :, QB : 2 * QB])
            for i in range(2):
                nc.scalar.dma_start(x8_sb[i][:, :, 2 * QB : Q], x8[i][:, :, 2 * QB : Q])

            for qb in range(NQB):
                # Phase-1 prologue for this q block: tT[e2, qb] = A.T @ x
                for e2 in range(ET):
                    pt = ps_mm.tile([P, QB], F32, name="ps_s")
                    for pr in range(2):
                        nc.tensor.matmul(
                            pt[:],
                            A8_sb[pr][:, :, e2 * P : (e2 + 1) * P],
                            x8_sb[pr][:, :, qb * QB : (qb + 1) * QB],
                            start=(pr == 0),
                            stop=(pr == 1),
                            perf_mode=DR,
                        )
                    nc.vector.tensor_copy(
                        t8_sb[e2 // 2][:, e2 % 2, qb * QB : (qb + 1) * QB], pt[:]
                    )

                att_ps = [ps_att.tile([P, E], F32, name=f"att{j}") for j in range(NQS)]
                den_ps = ps_den.tile([P, NQS], F32, name="den")
                p8_tiles = [None] * KP
                last = qb == NQB - 1
                for i in range(KP + PIPE):
                    if i < KP:
                        p8 = wpool.tile([P, 2, QB], FP8E4, name="p8")
                        p8_tiles[i] = p8
                        for half in range(2):
                            kt = 2 * i + half
                            st = ps_mm.tile([P, QB], F32, name="ps_s")
                            for pr in range(2):
                                nc.tensor.matmul(
                                    st[:],
                                    y8_sb[pr][:, :, kt * P : (kt + 1) * P],
                                    t8_sb[pr][:, :, qb * QB : (qb + 1) * QB],
                                    start=(pr == 0),
                                    stop=(pr == 1),
                                    perf_mode=DR,
                                )
                            nc.scalar.activation(
                                p8[:, half, :], st[:], exp, bias=bias_sb[:], scale=SCALE
                            )
                    if i >= PIPE:
                        kp = i - PIPE
                        p8p = p8_tiles[kp]
                        p8_tiles[kp] = None
                        # on the last kp of the last q block, finish den first
                        # so the reciprocal/epilogue chain starts earlier
                        den_first = last and kp == KP - 1
                        for j in range(NQS):
                            if den_first:
                                nc.tensor.matmul(
                                    den_ps[:, j : j + 1],
                                    p8p[:, :, j * QS : (j + 1) * QS],
                                    ones_ap,
                                    start=(kp == 0),
                                    stop=(kp == KP - 1),
                                    perf_mode=DR,
                                )
                        for j in range(NQS):
                            nc.tensor.matmul(
                                att_ps[j][:],
                                p8p[:, :, j * QS : (j + 1) * QS],
                                Vp8_sb[kp][:],
                                start=(kp == 0),
                                stop=(kp == KP - 1),
                                perf_mode=DR,
                            )
                            if not den_first:
                                nc.tensor.matmul(
                                    den_ps[:, j : j + 1],
                                    p8p[:, :, j * QS : (j + 1) * QS],
                                    ones_ap,
                                    start=(kp == 0),
                                    stop=(kp == KP - 1),
                                    perf_mode=DR,
                                )
                    if qb == 0 and i < KP:
                        # Vp[kt] = y @ WvoT for this kp, interleaved so the PE
                        # never sits idle in a separate phase. Emitted after
                        # att so the ps_mm rotation keeps >=1 iteration of
                        # slack before each psum tile is rewritten.
                        for half in range(2):
                            kt = 2 * i + half
                            pv = ps_mm.tile([P, E], F32, name="ps_s")
                            for pr in range(2):
                                nc.tensor.matmul(
                                    pv[:],
                                    y8_sb[pr][:, :, kt * P : (kt + 1) * P],
                                    Wv8_sb[pr][:],
                                    start=(pr == 0),
                                    stop=(pr == 1),
                                    perf_mode=DR,
                                )
                            nc.vector.tensor_copy(Vp8_sb[i][:, half, :], pv[:])

                rec_sb = opool.tile([P, NQS], F32, name="rec")
                nc.vector.reciprocal(rec_sb[:], den_ps[:])
                out_q = [nc.sync, nc.gpsimd, nc.sync, nc.scalar]
                for j in range(NQS):
                    o_sb = opool.tile([P, E], BF16, name="osb")
                    # alternate DVE/ACT so the last q-block's epilogue halves
                    if j % 2 == 0:
                        nc.vector.tensor_scalar_mul(
                            o_sb[:], att_ps[j][:], rec_sb[:, j : j + 1]
                        )
                    else:
                        nc.scalar.mul(o_sb[:], att_ps[j][:], rec_sb[:, j : j + 1])
                    eng = out_q[j] if last else (nc.sync if j % 2 == 0 else nc.gpsimd)
                    eng.dma_start(
                        out[qb * QB + j * QS : qb * QB + (j + 1) * QS, :], o_sb[:]
                    )

    _split_sync_waits(nc)
    return nc


_CACHED_NC = None


def _get_nc():
    global _CACHED_NC
    if _CACHED_NC is None:
        _CACHED_NC = _build_fp8()
    return _CACHED_NC


def _pair_pack(m):
    # [512, n] -> [2, 128, 2, n] with (pair, p, i) -> row pair*256 + i*128 + p
    n = m.shape[1]
    return np.ascontiguousarray(m.reshape(2, 2, P, n).transpose(0, 2, 1, 3))


def _prep_inputs(x, y, Wq, Wk, Wv, Wo):
    A8 = _pair_pack((Wq.T @ Wk).astype(E4_NP))
    WvoT8 = _pair_pack((Wv.T @ Wo.T).astype(E4_NP))
    x8 = np.stack([_pair_pack(x[n].T.astype(E4_NP)) for n in range(N_CORES)])
    y8 = np.stack([_pair_pack(y[n].T.astype(E4_NP)) for n in range(N_CORES)])
    return [
        {"x8": x8[n], "y8": y8[n], "A8": A8, "Wvo8": WvoT8} for n in range(N_CORES)
    ]


def run_device(x, y, Wq, Wk, Wv, Wo, **spmd_kwargs):
    nc = _get_nc()
    in_maps = _prep_inputs(x, y, Wq, Wk, Wv, Wo)
    res = run_bass_kernel_spmd(nc, in_maps, core_ids=list(range(N_CORES)), **spmd_kwargs)
    att = np.stack(
        [np.asarray(res.results[n]["out"]).astype(np.float32) for n in range(N_CORES)]
    )
    return att, res


def kernel(x, y, Wq, Wk, Wv, Wo, bo):
    x = np.asarray(x, dtype=np.float32)
    y = np.asarray(y, dtype=np.float32)
    Wq = np.asarray(Wq, dtype=np.float32)
    Wk = np.asarray(Wk, dtype=np.float32)
    Wv = np.asarray(Wv, dtype=np.float32)
    Wo = np.asarray(Wo, dtype=np.float32)
    bo = np.asarray(bo, dtype=np.float32)
    att, _ = run_device(x, y, Wq, Wk, Wv, Wo)
    return x + att + bo[None, None, :]


# revision 16
# speedup vs baseline: 1.1451x; 1.0328x over previous
"""Trainium2 Bass kernel for nn_CrossAttentionModule (head-collapsed cross attention).

Math (reference):
    Q = x @ Wq.T ; K = y @ Wk.T ; V = y @ Wv.T          (torch Linear convention)
    energy[n,q,k] = sum_{h,d} Q[n,q,h,d] K[n,k,h,d]     (heads summed!)
    att = softmax(energy / sqrt(512), axis=k)
    out = x + (att @ V) @ Wo.T + bo

Because heads are summed, energy = x @ (Wq.T @ Wk) @ y.T and the output
projection folds into V:  (att @ V) @ Wo.T = att @ (y @ (Wo @ Wv).T).
So we precompute on host (512x512, trivial):
    A    = Wq.T @ Wk        -> energy = (x @ A) @ y.T
    WvoT = Wv.T @ Wo.T      -> Vp = y @ WvoT ; att_out = att @ Vp
Device (per core, data-parallel over the N=8 batch), all fp8 DoubleRow:
    per q-block qb (512 wide):
      tT = A.T @ xT          [e2, qb]  (prologue, 8 MMs)
      kp loop (depth-2 software pipeline; on qb==0 also Vp[kt] = y @ WvoT):
        S^T tile = yT.T @ tT [k, q]   fp32 psum  (k on partitions)
        P = exp(S^T/sqrt(512) - C)    fp8
        att_psum += P.T @ Vp [q, f]   accumulated over kp
        den_tmp   = P.T @ ones        (transient psum; DVE-accumulated in SBUF)
      out_bf16 = att_psum * (1/den)  -> DRAM (per 128-row slice)
Host adds the residual x + out + bo in fp32.
"""

import sys

sys.path.insert(0, "/opt/trn_rl_repo")

import ml_dtypes
import numpy as np

import bass_rust
import concourse.bass as bass
import concourse.mybir as mybir
import concourse.tile as tile
from concourse.bass_utils import run_bass_kernel_spmd
from concourse.vector_clock import ScopedClock

N_CORES = 8
E = 512  # embed dim
Q = 2048  # query length (per batch element)
K = 4096  # key/value length
P = 128  # partitions
ET = E // P  # 4 embed tiles
QB = 512  # q block width for S^T matmuls
NQB = Q // QB  # 4
QS = P  # q sub-block (att psum partition dim)
NQS = QB // QS  # 4
KT = K // P  # 32 k tiles
KP = KT // 2  # 16 k-pair tiles (fp8 DoubleRow)
PIPE = 2  # kp-loop software pipeline depth
SCALE = float(1.0 / np.sqrt(np.float32(512.0)))
# exp shift: P' = exp(s/sqrt(512) - C) fits e4m3 (max logit ~8.1 -> P' <= 62);
# the flushed tail (weights < 2^-9 of e^C) carries ~1e-3 of the softmax mass.
C_SHIFT = 4.0

BF16 = mybir.dt.bfloat16
F32 = mybir.dt.float32
FP8E4 = mybir.dt.float8e4
BF16_NP = ml_dtypes.bfloat16
E4_NP = ml_dtypes.float8_e4m3


def _patched_drain_and_barrier(self, tick_clock, wait_clock):
    # The walrus build in this container caps sync-wait commands per CTRL
    # instruction below what Tile's tail drain emits; split the waits across
    # separate SP nops (same engine => same ordering semantics).
    nc = self.nc
    probe = nc.sync.nop(nofuse=True)
    wait_clock.add_sem_waits(probe.ins, ScopedClock({None: tick_clock.global_clock}))
    waits = list(probe.ins.sync_info.on_wait)
    probe.ins.sync_info = bass_rust.SyncInfo(on_wait=waits[:1], on_update=[])
    for wval in waits[1:]:
        n2 = nc.sync.nop(nofuse=True)
        n2.ins.sync_info = bass_rust.SyncInfo(on_wait=[wval], on_update=[])
    nc.sync.drain()
    nc.all_engine_barrier()
    popped = nc._tile_sem_poison_stack.pop()
    assert popped is self._sem_poison
    # Inline clear_and_free_semaphores, but spread the sem clears over all
    # engines (they serialize ~30ns each; ~250 sems on one engine is ~7us of
    # tail). dma_reset must stay on gpsimd. No trailing all_engine_barrier:
    # NEFF completion waits for every engine to halt anyway, so the next
    # execution still sees cleared semaphores.
    from concourse.bass import compact_to_ranges

    sems = list(self.sems.allocated().values())
    if sems:
        sem_nums = [s.num if hasattr(s, "num") else s for s in sems]
        engines = [nc.gpsimd, nc.vector, nc.scalar, nc.tensor, nc.sync]
        for sem_range in compact_to_ranges(sem_nums):
            assert nc._state.free_isdisjoint(sem_range)
            nc.gpsimd.dma_reset(sem_range)
            n = len(sem_range)
            n_eng = len(engines)
            step = (n + n_eng - 1) // n_eng
            for ei, lo in enumerate(range(0, n, step)):
                sub = range(sem_range.start + lo, sem_range.start + min(lo + step, n))
                engines[ei % n_eng].sem_clear(sub)
        nc._state.prepend_free_semaphores(sem_nums)
        for poison_set in nc._tile_sem_poison_stack:
            poison_set.update(sem_nums)


tile.TileContext._drain_and_barrier = _patched_drain_and_barrier

_MAX_WAITS = 1  # walrus merges Ldweights+Matmult waits into one struct capped at 2


def _split_sync_waits(nc, max_waits=_MAX_WAITS):
    # Hoist sem waits beyond the per-instruction cap onto same-engine NoOps
    # inserted right before the offender (same engine => same order semantics).
    # For Matmult preceded by its Ldweights, nops go before the Ldweights so
    # walrus can still fuse the pair (their waits are summed in the MM struct).
    n_nops = 0
    for f in nc.m.functions:
        for bb in f.blocks:
            new_insts = []
            changed = False
            for inst in bb.instructions:
                si = getattr(inst, "sync_info", None)
                waits = list(si.on_wait) if si is not None else []
                if len(waits) > max_waits:
                    head, rest = waits[:-max_waits], waits[-max_waits:]
                    pos = len(new_insts)
                    if (
                        isinstance(inst, mybir.InstMatmult)
                        and new_insts
                        and isinstance(new_insts[-1], mybir.InstLdweights)
                    ):
                        pos -= 1
                    nops = []
                    for i0 in range(0, len(head), max_waits):
                        nops.append(
                            mybir.InstNoOp(
                                name=f"{inst.name}-wsplit{i0}",
                                sync_info=mybir.SyncInfo(
                                    on_wait=head[i0 : i0 + max_waits], on_update=[]
                                ),
                                bass_nofuse=True,
                                engine=inst.engine,
                            )
                        )
                        n_nops += 1
                    new_insts[pos:pos] = nops
                    inst.sync_info = mybir.SyncInfo(
                        on_wait=rest, on_update=list(si.on_update)
                    )
                    changed = True
                new_insts.append(inst)
            if changed:
                bb.instructions = new_insts
    return n_nops


def _build_fp8():
    """fp8 DoubleRow variant: contraction dims pair-packed as [128, 2, n].

    Pair layout: virtual contraction row (pair, p, i) = index pair*256 + i*128 + p.
    lhsT and rhs use the same (p, i) mapping, so the DoubleRow pairing is
    consistent regardless of the hardware's internal interleave order.
    """
    nc = bass.Bass()
    # x8 chunked per (pr, qb): each [P, 2, QB] chunk contiguous in DRAM.
    # y8 chunked per (pr, quarter): each [P, 2, K//4] chunk contiguous.
    x8 = nc.dram_tensor("x8", [2, NQB, P, 2, QB], FP8E4, kind="ExternalInput")
    y8 = nc.dram_tensor("y8", [2, 4, P, 2, K // 4], FP8E4, kind="ExternalInput")
    A8 = nc.dram_tensor("A8", [2, P, 2, E], FP8E4, kind="ExternalInput")
    Wvo8 = nc.dram_tensor("Wvo8", [2, P, 2, E], FP8E4, kind="ExternalInput")
    out = nc.dram_tensor("out", [Q, E], BF16, kind="ExternalOutput")

    exp = mybir.ActivationFunctionType.Exp
    DR = mybir.MatmulPerfMode.DoubleRow
    QK = K // 4  # 1024 columns per y8 quarter chunk
    KTQ = QK // P  # 8 k tiles per quarter

    with tile.TileContext(nc) as tc:
        with (
            tc.tile_pool(name="const", bufs=1) as cpool,
            tc.tile_pool(name="pwork", bufs=4) as wpool,
            tc.tile_pool(name="outp", bufs=4) as opool,
            tc.tile_pool(name="ps_mm", bufs=3, space="PSUM") as ps_mm,
            tc.tile_pool(name="ps_att", bufs=1, space="PSUM") as ps_att,
            tc.tile_pool(name="ps_den", bufs=1, space="PSUM") as ps_den,
        ):
            x8_sb = [
                [cpool.tile([P, 2, QB], FP8E4, name=f"x8_{i}_{qb}") for qb in range(NQB)]
                for i in range(2)
            ]
            A8_sb = [cpool.tile([P, 2, E], FP8E4, name=f"A8{i}") for i in range(2)]
            y8_sb = [
                [cpool.tile([P, 2, QK], FP8E4, name=f"y8_{i}_{qt}") for qt in range(4)]
                for i in range(2)
            ]
            Wv8_sb = [cpool.tile([P, 2, E], FP8E4, name=f"Wv8{i}") for i in range(2)]
            t8_sb = [cpool.tile([P, 2, Q], FP8E4, name=f"t8{i}") for i in range(2)]
            Vp8_sb = [cpool.tile([P, 2, E], FP8E4, name=f"Vp8{i}") for i in range(KP)]
            ones_sb = cpool.tile([P, 32], FP8E4, name="ones")
            nc.vector.memset(ones_sb[:], 1.0)
            bias_sb = cpool.tile([P, 1], F32, name="biasC")
            nc.vector.memset(bias_sb[:], -C_SHIFT)
            # rhs AP [128, 2, 1] with middle step 16 (DoubleRow needs step%16==0)
            ones_ap = ones_sb.rearrange("p (i c) -> p i c", c=16)[:, :, 0:1]
            scratch_sb = cpool.tile([P, 1], FP8E4, name="scratch")
            # touch Exp early so the ACT table load (~1.3us) overlaps the
            # input DMAs instead of stalling the first real activation
            nc.scalar.activation(scratch_sb[:], bias_sb[:], exp, scale=1.0)

            def y8_kt(pr, kt):
                # lhsT slice for k tile kt out of the quarter-chunked y8
                return y8_sb[pr][kt // KTQ][:, :, (kt % KTQ) * P : (kt % KTQ + 1) * P]

            # Input DMAs: every transfer is a contiguous DRAM chunk. Rings are
            # scheduled in consumption order: A8 + x8-qb0 gate the first
            # prologue matmul, y8 q0 + Wv8 gate the first kp iteration.
            nc.scalar.dma_start(A8_sb[0][:], A8[0])
            nc.sync.dma_start(A8_sb[1][:], A8[1])
            nc.gpsimd.dma_start(x8_sb[0][0][:], x8[0, 0])
            nc.gpsimd.dma_start(x8_sb[1][0][:], x8[1, 0])
            nc.scalar.dma_start(y8_sb[0][0][:], y8[0, 0])
            nc.sync.dma_start(y8_sb[1][0][:], y8[1, 0])
            nc.scalar.dma_start(Wv8_sb[0][:], Wvo8[0])
            nc.sync.dma_start(Wv8_sb[1][:], Wvo8[1])
            for qtr in range(1, 4):
                nc.scalar.dma_start(y8_sb[0][qtr][:], y8[0, qtr])
                nc.sync.dma_start(y8_sb[1][qtr][:], y8[1, qtr])
            for qb in range(1, NQB):
                for i in range(2):
                    nc.gpsimd.dma_start(x8_sb[i][qb][:], x8[i, qb])

            def emit_p1(qb):
                # Phase-1 prologue for q block qb: tT[e2, qb] = A.T @ x.
                # The t8 casts go on DVE; callers emit this before any
                # epilogue muls so the casts aren't queued behind them.
                for e2 in range(ET):
                    pt = ps_mm.tile([P, QB], F32, name="ps_s")
                    for pr in range(2):
                        nc.tensor.matmul(
                            pt[:],
                            A8_sb[pr][:, :, e2 * P : (e2 + 1) * P],
                            x8_sb[pr][qb][:],
                            start=(pr == 0),
                            stop=(pr == 1),
                            perf_mode=DR,
                        )
                    nc.vector.tensor_copy(
                        t8_sb[e2 // 2][:, e2 % 2, qb * QB : (qb + 1) * QB], pt[:]
                    )

            emit_p1(0)
            for qb in range(NQB):
                att_ps = [ps_att.tile([P, E], F32, name=f"att{j}") for j in range(NQS)]
                den_ps = ps_den.tile([P, NQS], F32, name="den")
                p8_tiles = [None] * KP
                last = qb == NQB - 1
                for i in range(KP + PIPE):
                    if i < KP:
                        p8 = wpool.tile([P, 2, QB], FP8E4, name="p8")
                        p8_tiles[i] = p8
                        for half in range(2):
                            kt = 2 * i + half
                            st = ps_mm.tile([P, QB], F32, name="ps_s")
                            for pr in range(2):
                                nc.tensor.matmul(
                                    st[:],
                                    y8_kt(pr, kt),
                                    t8_sb[pr][:, :, qb * QB : (qb + 1) * QB],
                                    start=(pr == 0),
                                    stop=(pr == 1),
                                    perf_mode=DR,
                                )
                            nc.scalar.activation(
                                p8[:, half, :], st[:], exp, bias=bias_sb[:], scale=SCALE
                            )
                    if i >= PIPE:
                        kp = i - PIPE
                        p8p = p8_tiles[kp]
                        p8_tiles[kp] = None
                        if kp == KP - 1 and not last:
                            # hoist the next q block's phase-1 here, before the
                            # final att group: its t8 casts complete on DVE
                            # while the PE runs att(KP-1), so neither the next
                            # block's S^T nor this epilogue ever waits
                            emit_p1(qb + 1)
                        # on the last kp of the last q block, finish den first
                        # so the reciprocal/epilogue chain starts earlier
                        den_first = last and kp == KP - 1
                        for j in range(NQS):
                            if den_first:
                                nc.tensor.matmul(
                                    den_ps[:, j : j + 1],
                                    p8p[:, :, j * QS : (j + 1) * QS],
                                    ones_ap,
                                    start=(kp == 0),
                                    stop=(kp == KP - 1),
                                    perf_mode=DR,
                                )
                        for j in range(NQS):
                            nc.tensor.matmul(
                                att_ps[j][:],
                                p8p[:, :, j * QS : (j + 1) * QS],
                                Vp8_sb[kp][:],
                                start=(kp == 0),
                                stop=(kp == KP - 1),
                                perf_mode=DR,
                            )
                            if not den_first:
                                nc.tensor.matmul(
                                    den_ps[:, j : j + 1],
                                    p8p[:, :, j * QS : (j + 1) * QS],
                                    ones_ap,
                                    start=(kp == 0),
                                    stop=(kp == KP - 1),
                                    perf_mode=DR,
                                )
                    if qb == 0 and i < KP:
                        # Vp[kt] = y @ WvoT for this kp, interleaved so the PE
                        # never sits idle in a separate phase. Emitted after
                        # att so the ps_mm rotation keeps >=1 iteration of
                        # slack before each psum tile is rewritten.
                        for half in range(2):
                            kt = 2 * i + half
                            pv = ps_mm.tile([P, E], F32, name="ps_s")
                            for pr in range(2):
                                nc.tensor.matmul(
                                    pv[:],
                                    y8_kt(pr, kt),
                                    Wv8_sb[pr][:],
                                    start=(pr == 0),
                                    stop=(pr == 1),
                                    perf_mode=DR,
                                )
                            nc.vector.tensor_copy(Vp8_sb[i][:, half, :], pv[:])

                rec_sb = opool.tile([P, NQS], F32, name="rec")
                nc.vector.reciprocal(rec_sb[:], den_ps[:])
                for j in range(NQS):
                    o_sb = opool.tile([P, E], BF16, name="osb")
                    # alternate DVE/ACT so the last q-block's epilogue halves
                    if j % 2 == 0:
                        nc.vector.tensor_scalar_mul(
                            o_sb[:], att_ps[j][:], rec_sb[:, j : j + 1]
                        )
                        eng = nc.sync
                    else:
                        nc.scalar.mul(o_sb[:], att_ps[j][:], rec_sb[:, j : j + 1])
                        eng = nc.gpsimd
                    eng.dma_start(
                        out[qb * QB + j * QS : qb * QB + (j + 1) * QS, :], o_sb[:]
                    )

    _split_sync_waits(nc)
    return nc


_CACHED_NC = None


def _get_nc():
    global _CACHED_NC
    if _CACHED_NC is None:
        _CACHED_NC = _build_fp8()
    return _CACHED_NC


def _pair_pack(m):
    # [512, n] -> [2, 128, 2, n] with (pair, p, i) -> row pair*256 + i*128 + p
    n = m.shape[1]
    return np.ascontiguousarray(m.reshape(2, 2, P, n).transpose(0, 2, 1, 3))


def _chunk(m, csz):
    # [2, 128, 2, n] -> [2, n//csz, 128, 2, csz] with each chunk contiguous
    n = m.shape[-1]
    return np.ascontiguousarray(
        m.reshape(2, P, 2, n // csz, csz).transpose(0, 3, 1, 2, 4)
    )


def _prep_inputs(x, y, Wq, Wk, Wv, Wo):
    A8 = _pair_pack((Wq.T @ Wk).astype(E4_NP))
    WvoT8 = _pair_pack((Wv.T @ Wo.T).astype(E4_NP))
    x8 = np.stack([_chunk(_pair_pack(x[n].T.astype(E4_NP)), QB) for n in range(N_CORES)])
    y8 = np.stack(
        [_chunk(_pair_pack(y[n].T.astype(E4_NP)), K // 4) for n in range(N_CORES)]
    )
    return [
        {"x8": x8[n], "y8": y8[n], "A8": A8, "Wvo8": WvoT8} for n in range(N_CORES)
    ]


def run_device(x, y, Wq, Wk, Wv, Wo, **spmd_kwargs):
    nc = _get_nc()
    in_maps = _prep_inputs(x, y, Wq, Wk, Wv, Wo)
    res = run_bass_kernel_spmd(nc, in_maps, core_ids=list(range(N_CORES)), **spmd_kwargs)
    att = np.stack(
        [np.asarray(res.results[n]["out"]).astype(np.float32) for n in range(N_CORES)]
    )
    return att, res


def kernel(x, y, Wq, Wk, Wv, Wo, bo):
    x = np.asarray(x, dtype=np.float32)
    y = np.asarray(y, dtype=np.float32)
    Wq = np.asarray(Wq, dtype=np.float32)
    Wk = np.asarray(Wk, dtype=np.float32)
    Wv = np.asarray(Wv, dtype=np.float32)
    Wo = np.asarray(Wo, dtype=np.float32)
    bo = np.asarray(bo, dtype=np.float32)
    att, _ = run_device(x, y, Wq, Wk, Wv, Wo)
    return x + att + bo[None, None, :]


# revision 23
# speedup vs baseline: 1.1631x; 1.0157x over previous
"""Trainium2 Bass kernel for nn_CrossAttentionModule (head-collapsed cross attention).

Math (reference):
    Q = x @ Wq.T ; K = y @ Wk.T ; V = y @ Wv.T          (torch Linear convention)
    energy[n,q,k] = sum_{h,d} Q[n,q,h,d] K[n,k,h,d]     (heads summed!)
    att = softmax(energy / sqrt(512), axis=k)
    out = x + (att @ V) @ Wo.T + bo

Because heads are summed, energy = x @ (Wq.T @ Wk) @ y.T and the output
projection folds into V:  (att @ V) @ Wo.T = att @ (y @ (Wo @ Wv).T).
So we precompute on host (512x512, trivial):
    A    = Wq.T @ Wk        -> energy = (x @ A) @ y.T
    WvoT = Wv.T @ Wo.T      -> Vp = y @ WvoT ; att_out = att @ Vp
Device (per core, data-parallel over the N=8 batch), all fp8 DoubleRow:
    per q-block qb (512 wide):
      tT = A.T @ xT          [e2, qb]  (prologue, 8 MMs)
      kp loop (depth-2 software pipeline; on qb==0 also Vp[kt] = y @ WvoT):
        S^T tile = yT.T @ tT [k, q]   fp32 psum  (k on partitions)
        P = exp(S^T/sqrt(512) - C)    fp8
        att_psum += P.T @ Vp [q, f]   accumulated over kp
        den_tmp   = P.T @ ones        (transient psum; DVE-accumulated in SBUF)
      out_bf16 = att_psum * (1/den)  -> DRAM (per 128-row slice)
Host adds the residual x + out + bo in fp32.
"""

import sys

sys.path.insert(0, "/opt/trn_rl_repo")

import ml_dtypes
import numpy as np

import bass_rust
import concourse.bass as bass
import concourse.mybir as mybir
import concourse.tile as tile
from concourse.bass_utils import run_bass_kernel_spmd
from concourse.vector_clock import ScopedClock

N_CORES = 8
E = 512  # embed dim
Q = 2048  # query length (per batch element)
K = 4096  # key/value length
P = 128  # partitions
ET = E // P  # 4 embed tiles
QB = 512  # q block width for S^T matmuls
NQB = Q // QB  # 4
QS = P  # q sub-block (att psum partition dim)
NQS = QB // QS  # 4
KT = K // P  # 32 k tiles
KP = KT // 2  # 16 k-pair tiles (fp8 DoubleRow)
PIPE = 2  # kp-loop software pipeline depth
SCALE = float(1.0 / np.sqrt(np.float32(512.0)))
# exp shift: P' = exp(s/sqrt(512) - C) fits e4m3 (max logit ~8.1 -> P' <= 62);
# the flushed tail (weights < 2^-9 of e^C) carries ~1e-3 of the softmax mass.
C_SHIFT = 4.0

BF16 = mybir.dt.bfloat16
F32 = mybir.dt.float32
FP8E4 = mybir.dt.float8e4
BF16_NP = ml_dtypes.bfloat16
E4_NP = ml_dtypes.float8_e4m3


def _patched_drain_and_barrier(self, tick_clock, wait_clock):
    # The walrus build in this container caps sync-wait commands per CTRL
    # instruction below what Tile's tail drain emits; split the waits across
    # separate SP nops (same engine => same ordering semantics).
    nc = self.nc
    probe = nc.sync.nop(nofuse=True)
    wait_clock.add_sem_waits(probe.ins, ScopedClock({None: tick_clock.global_clock}))
    waits = list(probe.ins.sync_info.on_wait)
    probe.ins.sync_info = bass_rust.SyncInfo(on_wait=waits[:1], on_update=[])
    for wval in waits[1:]:
        n2 = nc.sync.nop(nofuse=True)
        n2.ins.sync_info = bass_rust.SyncInfo(on_wait=[wval], on_update=[])
    nc.sync.drain()
    nc.all_engine_barrier()
    popped = nc._tile_sem_poison_stack.pop()
    assert popped is self._sem_poison
    # Inline clear_and_free_semaphores, but spread the sem clears over all
    # engines (they serialize ~30ns each; ~250 sems on one engine is ~7us of
    # tail). dma_reset must stay on gpsimd. No trailing all_engine_barrier:
    # NEFF completion waits for every engine to halt anyway, so the next
    # execution still sees cleared semaphores.
    from concourse.bass import compact_to_ranges

    sems = list(self.sems.allocated().values())
    if sems:
        sem_nums = [s.num if hasattr(s, "num") else s for s in sems]
        engines = [nc.gpsimd, nc.vector, nc.scalar, nc.tensor, nc.sync]
        for sem_range in compact_to_ranges(sem_nums):
            assert nc._state.free_isdisjoint(sem_range)
            nc.gpsimd.dma_reset(sem_range)
            n = len(sem_range)
            n_eng = len(engines)
            step = (n + n_eng - 1) // n_eng
            for ei, lo in enumerate(range(0, n, step)):
                sub = range(sem_range.start + lo, sem_range.start + min(lo + step, n))
                engines[ei % n_eng].sem_clear(sub)
        nc._state.prepend_free_semaphores(sem_nums)
        for poison_set in nc._tile_sem_poison_stack:
            poison_set.update(sem_nums)


tile.TileContext._drain_and_barrier = _patched_drain_and_barrier

_MAX_WAITS = 1  # walrus merges Ldweights+Matmult waits into one struct capped at 2


def _split_sync_waits(nc, max_waits=_MAX_WAITS):
    # Hoist sem waits beyond the per-instruction cap onto same-engine NoOps
    # inserted right before the offender (same engine => same order semantics).
    # For Matmult preceded by its Ldweights, nops go before the Ldweights so
    # walrus can still fuse the pair (their waits are summed in the MM struct).
    n_nops = 0
    for f in nc.m.functions:
        for bb in f.blocks:
            new_insts = []
            changed = False
            for inst in bb.instructions:
                si = getattr(inst, "sync_info", None)
                waits = list(si.on_wait) if si is not None else []
                if len(waits) > max_waits:
                    head, rest = waits[:-max_waits], waits[-max_waits:]
                    pos = len(new_insts)
                    if (
                        isinstance(inst, mybir.InstMatmult)
                        and new_insts
                        and isinstance(new_insts[-1], mybir.InstLdweights)
                    ):
                        pos -= 1
                    nops = []
                    for i0 in range(0, len(head), max_waits):
                        nops.append(
                            mybir.InstNoOp(
                                name=f"{inst.name}-wsplit{i0}",
                                sync_info=mybir.SyncInfo(
                                    on_wait=head[i0 : i0 + max_waits], on_update=[]
                                ),
                                bass_nofuse=True,
                                engine=inst.engine,
                            )
                        )
                        n_nops += 1
                    new_insts[pos:pos] = nops
                    inst.sync_info = mybir.SyncInfo(
                        on_wait=rest, on_update=list(si.on_update)
                    )
                    changed = True
                new_insts.append(inst)
            if changed:
                bb.instructions = new_insts
    return n_nops


def _build_fp8():
    """fp8 DoubleRow variant: contraction dims pair-packed as [128, 2, n].

    Pair layout: virtual contraction row (pair, p, i) = index pair*256 + i*128 + p.
    lhsT and rhs use the same (p, i) mapping, so the DoubleRow pairing is
    consistent regardless of the hardware's internal interleave order.
    """
    nc = bass.Bass()
    # All inputs chunked so every DMA is one contiguous DRAM read:
    # x8 per (pr, qb), y8 per (pr, kp), A8 per (pr, e2).
    x8 = nc.dram_tensor("x8", [2, NQB, P, 2, QB], FP8E4, kind="ExternalInput")
    y8 = nc.dram_tensor("y8", [2, KP, P, 2, 2 * P], FP8E4, kind="ExternalInput")
    A8 = nc.dram_tensor("A8", [2, ET, P, 2, P], FP8E4, kind="ExternalInput")
    Wvo8 = nc.dram_tensor("Wvo8", [2, P, 2, E], FP8E4, kind="ExternalInput")
    out = nc.dram_tensor("out", [Q, E], BF16, kind="ExternalOutput")

    exp = mybir.ActivationFunctionType.Exp
    DR = mybir.MatmulPerfMode.DoubleRow
    N_WARM = 44  # dummy PE warm-up matmuls issued during the input DMA wait

    with tile.TileContext(nc) as tc:
        with (
            tc.tile_pool(name="const", bufs=1) as cpool,
            tc.tile_pool(name="pwork", bufs=4) as wpool,
            tc.tile_pool(name="outp", bufs=4) as opool,
            tc.tile_pool(name="ps_mm", bufs=3, space="PSUM") as ps_mm,
            tc.tile_pool(name="ps_att", bufs=1, space="PSUM") as ps_att,
            tc.tile_pool(name="ps_den", bufs=1, space="PSUM") as ps_den,
        ):
            x8_sb = [
                [cpool.tile([P, 2, QB], FP8E4, name=f"x8_{i}_{qb}") for qb in range(NQB)]
                for i in range(2)
            ]
            A8_sb = [
                [cpool.tile([P, 2, P], FP8E4, name=f"A8_{i}_{e}") for e in range(ET)]
                for i in range(2)
            ]
            y8_sb = [
                [cpool.tile([P, 2, 2 * P], FP8E4, name=f"y8_{i}_{kp}") for kp in range(KP)]
                for i in range(2)
            ]
            Wv8_sb = [cpool.tile([P, 2, E], FP8E4, name=f"Wv8{i}") for i in range(2)]
            t8_sb = [cpool.tile([P, 2, Q], FP8E4, name=f"t8{i}") for i in range(2)]
            Vp8_sb = [cpool.tile([P, 2, E], FP8E4, name=f"Vp8{i}") for i in range(KP)]
            ones_sb = cpool.tile([P, 32], FP8E4, name="ones")
            nc.vector.memset(ones_sb[:], 1.0)
            bias_sb = cpool.tile([P, 1], F32, name="biasC")
            nc.vector.memset(bias_sb[:], -C_SHIFT)
            # rhs AP [128, 2, 1] with middle step 16 (DoubleRow needs step%16==0)
            ones_ap = ones_sb.rearrange("p (i c) -> p i c", c=16)[:, :, 0:1]
            ones_dr = ones_sb.rearrange("p (i c) -> p i c", c=16)
            scratch_sb = cpool.tile([P, 1], FP8E4, name="scratch")
            # touch Exp early so the ACT table load (~1.3us) overlaps the
            # input DMAs instead of stalling the first real activation
            nc.scalar.activation(scratch_sb[:], bias_sb[:], exp, scale=1.0)

            def y8_kt(pr, kt):
                # lhsT slice for k tile kt out of the kp-chunked y8
                return y8_sb[pr][kt // 2][:, :, (kt % 2) * P : (kt % 2 + 1) * P]

            # Input DMAs: every transfer is one contiguous DRAM chunk, issued
            # in consumption order across the three DMA-capable rings
            # (~55 GB/s each): A8 e2 chunks + x8-qb0 gate the phase-1
            # prologue, y8 kp0 + Wv8 gate the first kp iterations.
            nc.scalar.dma_start(A8_sb[0][0][:], A8[0, 0])
            nc.sync.dma_start(x8_sb[1][0][:], x8[1, 0])
            nc.gpsimd.dma_start(x8_sb[0][0][:], x8[0, 0])
            nc.scalar.dma_start(A8_sb[1][0][:], A8[1, 0])
            nc.scalar.dma_start(A8_sb[0][1][:], A8[0, 1])
            nc.scalar.dma_start(A8_sb[1][1][:], A8[1, 1])
            for e2 in range(2, ET):
                for i in range(2):
                    nc.sync.dma_start(A8_sb[i][e2][:], A8[i, e2])
            nc.gpsimd.dma_start(y8_sb[0][0][:], y8[0, 0])
            nc.gpsimd.dma_start(y8_sb[1][0][:], y8[1, 0])
            nc.gpsimd.dma_start(Wv8_sb[0][:], Wvo8[0])
            nc.sync.dma_start(Wv8_sb[1][:], Wvo8[1])
            # early y8 kps spread over all rings; the bulk tail on sync
            for kp, eng in ((1, nc.scalar), (2, nc.gpsimd), (3, nc.sync), (4, nc.gpsimd), (5, nc.scalar)):
                eng.dma_start(y8_sb[0][kp][:], y8[0, kp])
                eng.dma_start(y8_sb[1][kp][:], y8[1, kp])
            for kp in range(6, KP):
                for i in range(2):
                    nc.sync.dma_start(y8_sb[i][kp][:], y8[i, kp])
            for qb in range(1, 3):
                for i in range(2):
                    nc.gpsimd.dma_start(x8_sb[i][qb][:], x8[i, qb])
            for i in range(2):
                nc.sync.dma_start(x8_sb[i][3][:], x8[i, 3])

            # PE warm-up: tiny dummy matmuls with no data dependencies keep
            # the PE busy through the input-DMA wait so the HAM clock gate
            # reaches 8/8 before real work starts (saves ~2us of cold rate).
            warm_ps = ps_den.tile([16, 16], F32, name="den")
            for _ in range(N_WARM):
                nc.tensor.matmul(
                    warm_ps[:], ones_dr[:, :, 0:16], ones_dr[:, :, 0:16],
                    start=True, stop=True, perf_mode=DR,
                )

            def emit_p1(qb):
                # Phase-1 prologue for q block qb: tT[e2, qb] = A.T @ x.
                # The t8 casts go on DVE; callers emit this before any
                # epilogue muls so the casts aren't queued behind them.
                for e2 in range(ET):
                    pt = ps_mm.tile([P, QB], F32, name="ps_s")
                    for pr in range(2):
                        nc.tensor.matmul(
                            pt[:],
                            A8_sb[pr][e2][:],
                            x8_sb[pr][qb][:],
                            start=(pr == 0),
                            stop=(pr == 1),
                            perf_mode=DR,
                        )
                    nc.vector.tensor_copy(
                        t8_sb[e2 // 2][:, e2 % 2, qb * QB : (qb + 1) * QB], pt[:]
                    )

            emit_p1(0)
            for qb in range(NQB):
                att_ps = [ps_att.tile([P, E], F32, name=f"att{j}") for j in range(NQS)]
                den_ps = ps_den.tile([P, NQS], F32, name="den")
                p8_tiles = [None] * KP
                last = qb == NQB - 1
                for i in range(KP + PIPE):
                    if i < KP:
                        p8 = wpool.tile([P, 2, QB], FP8E4, name="p8")
                        p8_tiles[i] = p8
                        for half in range(2):
                            kt = 2 * i + half
                            st = ps_mm.tile([P, QB], F32, name="ps_s")
                            for pr in range(2):
                                nc.tensor.matmul(
                                    st[:],
                                    y8_kt(pr, kt),
                                    t8_sb[pr][:, :, qb * QB : (qb + 1) * QB],
                                    start=(pr == 0),
                                    stop=(pr == 1),
                                    perf_mode=DR,
                                )
                            nc.scalar.activation(
                                p8[:, half, :], st[:], exp, bias=bias_sb[:], scale=SCALE
                            )
                    if i >= PIPE:
                        kp = i - PIPE
                        p8p = p8_tiles[kp]
                        p8_tiles[kp] = None
                        # on the last kp of the last q block, finish den first
                        # so the reciprocal/epilogue chain starts earlier
                        den_first = last and kp == KP - 1
                        for j in range(NQS):
                            if den_first:
                                nc.tensor.matmul(
                                    den_ps[:, j : j + 1],
                                    p8p[:, :, j * QS : (j + 1) * QS],
                                    ones_ap,
                                    start=(kp == 0),
                                    stop=(kp == KP - 1),
                                    perf_mode=DR,
                                )
                        for j in range(NQS):
                            nc.tensor.matmul(
                                att_ps[j][:],
                                p8p[:, :, j * QS : (j + 1) * QS],
                                Vp8_sb[kp][:],
                                start=(kp == 0),
                                stop=(kp == KP - 1),
                                perf_mode=DR,
                            )
                            if not den_first:
                                nc.tensor.matmul(
                                    den_ps[:, j : j + 1],
                                    p8p[:, :, j * QS : (j + 1) * QS],
                                    ones_ap,
                                    start=(kp == 0),
                                    stop=(kp == KP - 1),
                                    perf_mode=DR,
                                )
                    if qb == 0 and i < KP:
                        # Vp[kt] = y @ WvoT for this kp, interleaved so the PE
                        # never sits idle in a separate phase. Emitted after
                        # att so the ps_mm rotation keeps >=1 iteration of
                        # slack before each psum tile is rewritten.
                        for half in range(2):
                            kt = 2 * i + half
                            pv = ps_mm.tile([P, E], F32, name="ps_s")
                            for pr in range(2):
                                nc.tensor.matmul(
                                    pv[:],
                                    y8_kt(pr, kt),
                                    Wv8_sb[pr][:],
                                    start=(pr == 0),
                                    stop=(pr == 1),
                                    perf_mode=DR,
                                )
                            nc.vector.tensor_copy(Vp8_sb[i][:, half, :], pv[:])
                    if i == KP - 1 and not last:
                        # hoist the next q block's phase-1 prologue two
                        # iterations before the boundary: its t8 casts drain
                        # on DVE during att(KP-2..KP-1), so neither the next
                        # block's S^T nor this block's epilogue ever waits
                        emit_p1(qb + 1)

                rec_sb = opool.tile([P, NQS], F32, name="rec")
                nc.vector.reciprocal(rec_sb[:], den_ps[:])
                for j in range(NQS):
                    o_sb = opool.tile([P, E], BF16, name="osb")
                    # Mid-kernel the muls all go on DVE so ACT stays free for
                    # the next block's exps; only the last block (nothing left
                    # to protect) splits DVE/ACT to halve the tail.
                    if last and j % 2 == 1:
                        nc.scalar.mul(o_sb[:], att_ps[j][:], rec_sb[:, j : j + 1])
                        eng = nc.gpsimd
                    else:
                        nc.vector.tensor_scalar_mul(
                            o_sb[:], att_ps[j][:], rec_sb[:, j : j + 1]
                        )
                        eng = nc.sync if j % 2 == 0 else nc.gpsimd
                    eng.dma_start(
                        out[qb * QB + j * QS : qb * QB + (j + 1) * QS, :], o_sb[:]
                    )

    _split_sync_waits(nc)
    return nc


_CACHED_NC = None


def _get_nc():
    global _CACHED_NC
    if _CACHED_NC is None:
        _CACHED_NC = _build_fp8()
    return _CACHED_NC


def _pair_pack(m):
    # [512, n] -> [2, 128, 2, n] with (pair, p, i) -> row pair*256 + i*128 + p
    n = m.shape[1]
    return np.ascontiguousarray(m.reshape(2, 2, P, n).transpose(0, 2, 1, 3))


def _chunk(m, csz):
    # [2, 128, 2, n] -> [2, n//csz, 128, 2, csz] with each chunk contiguous
    n = m.shape[-1]
    return np.ascontiguousarray(
        m.reshape(2, P, 2, n // csz, csz).transpose(0, 3, 1, 2, 4)
    )


def _prep_inputs(x, y, Wq, Wk, Wv, Wo):
    A8 = _chunk(_pair_pack((Wq.T @ Wk).astype(E4_NP)), P)
    WvoT8 = _pair_pack((Wv.T @ Wo.T).astype(E4_NP))
    x8 = np.stack([_chunk(_pair_pack(x[n].T.astype(E4_NP)), QB) for n in range(N_CORES)])
    y8 = np.stack(
        [_chunk(_pair_pack(y[n].T.astype(E4_NP)), 2 * P) for n in range(N_CORES)]
    )
    return [
        {"x8": x8[n], "y8": y8[n], "A8": A8, "Wvo8": WvoT8} for n in range(N_CORES)
    ]


def run_device(x, y, Wq, Wk, Wv, Wo, **spmd_kwargs):
    nc = _get_nc()
    in_maps = _prep_inputs(x, y, Wq, Wk, Wv, Wo)
    res = run_bass_kernel_spmd(nc, in_maps, core_ids=list(range(N_CORES)), **spmd_kwargs)
    att = np.stack(
        [np.asarray(res.results[n]["out"]).astype(np.float32) for n in range(N_CORES)]
    )
    return att, res


def kernel(x, y, Wq, Wk, Wv, Wo, bo):
    x = np.asarray(x, dtype=np.float32)
    y = np.asarray(y, dtype=np.float32)
    Wq = np.asarray(Wq, dtype=np.float32)
    Wk = np.asarray(Wk, dtype=np.float32)
    Wv = np.asarray(Wv, dtype=np.float32)
    Wo = np.asarray(Wo, dtype=np.float32)
    bo = np.asarray(bo, dtype=np.float32)
    att, _ = run_device(x, y, Wq, Wk, Wv, Wo)
    return x + att + bo[None, None, :]
